# revision 2
# baseline (speedup 1.0000x reference)
"""Additive-attention kernel for one TRN2 chip (8 NeuronCores, data parallel).

Computes, per batch element b:
    e      = tanh(features @ W_feat + hidden @ W_hidden + b_feat + b_hidden)
    scores = e @ V_w                (V_b dropped: softmax is shift-invariant)
    alpha  = softmax(scores, axis=regions)
    context= sum_r alpha[b,r] * features[b,r,:]

Device strategy (per core, 64 batches):
  - Host pre-packs X'^T = [features; hidden]^T in bf16 with regions padded
    49->50, so the contraction dim (2304 = 2048+256) sits on partitions.
  - Stage 1 on TensorE: e^T[a, row] accumulated over 18 K-chunks; tanh+bias
    fused into the PSUM eviction on ScalarE.
  - Scores via matmul with V as stationary; softmax on an [8, 50] tile.
  - alpha broadcast to 128 partitions via a K=1 matmul with a ones vector.
  - Stage 2 on VectorE: context^T[f, b] = reduce_r(X^T * alpha_bcast) using
    the same X^T tiles (no second layout of features needed).
  - context^T transposed back via PE at the end; outputs ctx f32 / alpha f32.
"""

import numpy as np
import ml_dtypes

import concourse.bass as bass
import concourse.tile as tile
import concourse.mybir as mybir
from concourse import bacc
from concourse.bass_utils import run_bass_kernel_spmd
from concourse.masks import make_identity

# Problem shapes (hardcoded per contract)
B, R, F, H, A = 512, 49, 2048, 256, 256
NCORES = 8
B_LOC = B // NCORES      # 64 batches per core
R_PAD = 50               # regions padded for alignment
ROWS = B_LOC * R_PAD     # 3200
K = F + H                # 2304 contraction (features + hidden fold-in)
KC = K // 128            # 18 K-chunks
FC = F // 128            # 16 feature chunks (stage 2 uses only these)
BLK_B = 8                # batches per pipeline block
BLK_ROWS = BLK_B * R_PAD # 400 = moving free dim
NBLK = B_LOC // BLK_B    # 8 blocks

BF16 = mybir.dt.bfloat16
F32 = mybir.dt.float32

_CACHE = {}


def _build_module():
    nc = bacc.Bacc(
        "TRN2",
        target_bir_lowering=False,
        debug=False,
        num_devices=NCORES,
    )
    xt = nc.dram_tensor("xt", [K, ROWS], BF16, kind="ExternalInput").ap()
    w = nc.dram_tensor("w", [K, A], BF16, kind="ExternalInput").ap()
    bias = nc.dram_tensor("bias", [128, 2], F32, kind="ExternalInput").ap()
    v = nc.dram_tensor("v", [128, 2], BF16, kind="ExternalInput").ap()
    ctx_out = nc.dram_tensor("ctx", [B_LOC, F], F32, kind="ExternalOutput").ap()
    alpha_out = nc.dram_tensor("alpha", [B_LOC, R], F32, kind="ExternalOutput").ap()

    with tile.TileContext(nc) as tc:
        _kernel_body(tc, xt, w, bias, v, ctx_out, alpha_out)
    nc.compile()
    return nc


def _kernel_body(tc, xt, w, bias, v, ctx_out, alpha_out):
    nc = tc.nc
    AX = mybir.AxisListType
    AF = mybir.ActivationFunctionType

    with (
        tc.tile_pool(name="const", bufs=1) as const_pool,
        tc.tile_pool(name="xtp", bufs=3) as xt_pool,
        tc.tile_pool(name="work", bufs=2) as work,
        tc.tile_pool(name="small", bufs=2) as small,
        tc.tile_pool(name="ps", bufs=2, space="PSUM") as psum,
    ):
        # ---- constants ----
        w_sb = const_pool.tile([128, KC, A], BF16)
        nc.sync.dma_start(w_sb[:], w.rearrange("(kc p) a -> p kc a", p=128))
        bias_sb = const_pool.tile([128, 2], F32)
        nc.sync.dma_start(bias_sb[:], bias)
        v_sb = const_pool.tile([128, 2], BF16)
        nc.sync.dma_start(v_sb[:], v)
        ones_sb = const_pool.tile([1, 128], BF16)
        nc.vector.memset(ones_sb[:], 1.0)
        ident = const_pool.tile([128, 128], BF16)
        make_identity(nc, ident[:])
        # context^T accumulator [f_in_chunk, (fc, b)]
        ctxT = const_pool.tile([128, FC, B_LOC], BF16)

        for blk in range(NBLK):
            xt_sb = xt_pool.tile([128, KC, BLK_ROWS], BF16)
            nc.sync.dma_start(
                xt_sb[:],
                xt.rearrange("(kc p) (nb n) -> p kc nb n", p=128, n=BLK_ROWS)[
                    :, :, blk, :
                ],
            )

            # ---- stage 1: e^T = tanh(W'^T X'^T + bias) ----
            e_sb = work.tile([128, 2, BLK_ROWS], BF16)
            for a0 in range(2):
                pe_ps = psum.tile([128, BLK_ROWS], F32, tag="pe", bufs=4)
                for kc in range(KC):
                    nc.tensor.matmul(
                        pe_ps[:],
                        lhsT=w_sb[:, kc, a0 * 128:(a0 + 1) * 128],
                        rhs=xt_sb[:, kc, :],
                        start=(kc == 0),
                        stop=(kc == KC - 1),
                    )
                nc.scalar.activation(
                    e_sb[:, a0, :], pe_ps[:], AF.Tanh,
                    bias=bias_sb[:, a0:a0 + 1], scale=1.0,
                )

            # ---- scores = V . e ----
            sc_ps = psum.tile([1, BLK_ROWS], F32, tag="sc", bufs=1)
            for a0 in range(2):
                nc.tensor.matmul(
                    sc_ps[:],
                    lhsT=v_sb[:, a0:a0 + 1],
                    rhs=e_sb[:, a0, :],
                    start=(a0 == 0),
                    stop=(a0 == 1),
                )
            sc_sb = small.tile([1, BLK_ROWS], F32)
            nc.scalar.copy(sc_sb[:], sc_ps[:])

            # ---- softmax over regions, on an [8, 50] layout ----
            sm = small.tile([BLK_B, R_PAD], F32)
            nc.sync.dma_start(sm[:], sc_sb[:])
            neg_mx = small.tile([BLK_B, 1], F32)
            nc.vector.tensor_reduce(
                neg_mx[:], sm[:, 0:R], axis=AX.X, op=mybir.AluOpType.max,
                negate=True,
            )
            ex = small.tile([BLK_B, R_PAD], F32)
            nc.scalar.activation(ex[:], sm[:], AF.Exp, bias=neg_mx[:], scale=1.0)
            den = small.tile([BLK_B, 1], F32)
            nc.vector.tensor_reduce(
                den[:], ex[:, 0:R], axis=AX.X, op=mybir.AluOpType.add
            )
            rec = small.tile([BLK_B, 1], F32)
            nc.vector.reciprocal(rec[:], den[:])
            al = small.tile([BLK_B, R], F32)
            nc.scalar.activation(al[:], ex[:, 0:R], AF.Copy, scale=rec[:])
            nc.sync.dma_start(alpha_out[blk * BLK_B:(blk + 1) * BLK_B, :], al[:])
            al_bf = small.tile([BLK_B, R_PAD], BF16)
            nc.vector.memset(al_bf[:, R:R_PAD], 0.0)
            nc.scalar.activation(al_bf[:, 0:R], ex[:, 0:R], AF.Copy, scale=rec[:])
            al_row = small.tile([1, BLK_ROWS], BF16)
            nc.sync.dma_start(al_row[:], al_bf[:])

            # ---- broadcast alpha across partitions via K=1 matmul ----
            ab_ps = psum.tile([128, BLK_ROWS], F32, tag="ab", bufs=1)
            nc.tensor.matmul(
                ab_ps[:], lhsT=ones_sb[:], rhs=al_row[:], start=True, stop=True
            )
            ab_sb = work.tile([128, BLK_ROWS], BF16)
            nc.scalar.copy(ab_sb[:], ab_ps[:])

            # ---- stage 2: context^T += sum_r alpha * X^T ----
            tmp = work.tile([128, FC, BLK_ROWS], BF16)
            nc.vector.tensor_mul(
                tmp[:],
                xt_sb[:, 0:FC, :],
                ab_sb.rearrange("p (x n) -> p x n", x=1).broadcast_to(
                    [128, FC, BLK_ROWS]
                ),
            )
            with nc.allow_low_precision("context emitted once in bf16"):
                nc.vector.tensor_reduce(
                    ctxT[:, :, blk * BLK_B:(blk + 1) * BLK_B],
                    tmp.rearrange("p fc (b r) -> p fc b r", b=BLK_B),
                    axis=AX.X,
                    op=mybir.AluOpType.add,
                )

        # ---- transpose context^T -> context and store ----
        ctx_nat = const_pool.tile([64, FC, 128], F32)
        for kc in range(FC):
            tp_ps = psum.tile([64, 128], BF16, tag="tp", bufs=2)
            nc.tensor.transpose(tp_ps[:], ctxT[:, kc, :], ident[:])
            nc.scalar.copy(ctx_nat[:, kc, :], tp_ps[:])
        nc.sync.dma_start(ctx_out[:], ctx_nat.rearrange("b fc f -> b (fc f)"))


def _prep_inputs(features, hidden, W_feat, b_feat, W_hidden, b_hidden, V_w, V_b):
    """Host-side shard + pack. Returns in_maps for the 8 cores."""
    bf = ml_dtypes.bfloat16
    w_all = np.concatenate([W_feat, W_hidden], axis=0).astype(bf)       # [K, A]
    bias_all = (b_feat + b_hidden).astype(np.float32)                   # [A]
    bias_pk = np.ascontiguousarray(bias_all.reshape(2, 128).T)          # [128, 2]
    v_pk = np.ascontiguousarray(V_w.astype(bf).reshape(2, 128).T)       # [128, 2]

    in_maps = []
    for c in range(NCORES):
        fs = features[c * B_LOC:(c + 1) * B_LOC]    # [64, 49, 2048] f32
        hs = hidden[c * B_LOC:(c + 1) * B_LOC]      # [64, 256] f32
        xt = np.zeros((K, B_LOC, R_PAD), dtype=bf)
        xt[:F, :, :R] = fs.transpose(2, 0, 1).astype(bf)
        xt[F:, :, :] = hs.T.astype(bf)[:, :, None]
        in_maps.append({
            "xt": np.ascontiguousarray(xt.reshape(K, ROWS)),
            "w": w_all,
            "bias": bias_pk,
            "v": v_pk,
        })
    return in_maps


def run(inputs, trace=False):
    """Compile (cached), execute on 8 cores, gather. Returns dict with
    full-shape outputs and profiling info."""
    if "nc" not in _CACHE:
        _CACHE["nc"] = _build_module()
    nc = _CACHE["nc"]
    in_maps = _prep_inputs(**inputs)
    res = run_bass_kernel_spmd(
        nc, in_maps, core_ids=list(range(NCORES)), trace=trace
    )
    ctx = np.concatenate([r["ctx"] for r in res.results], axis=0)
    alpha = np.concatenate([r["alpha"] for r in res.results], axis=0)
    return {
        "context": ctx.astype(np.float32),
        "alpha": alpha.astype(np.float32),
        "exec_time_ns": res.exec_time_ns,
        "profile_json": res.profile_json,
    }


def kernel(**inputs):
    out = run(inputs, trace=False)
    return out["context"], out["alpha"]


# revision 6
# speedup vs baseline: 1.1814x; 1.1814x over previous
"""Additive-attention kernel for one TRN2 chip (8 NeuronCores, data parallel).

Computes, per batch element b:
    e      = tanh(features @ W_feat + hidden @ W_hidden + b_feat + b_hidden)
    scores = e @ V_w                (V_b dropped: softmax is shift-invariant)
    alpha  = softmax(scores, axis=regions)
    context= sum_r alpha[b,r] * features[b,r,:]

Device strategy (per core, 64 batches):
  - Host pre-packs X'^T = [features; hidden]^T in bf16 with regions padded
    49->50, so the contraction dim (2304 = 2048+256) sits on partitions.
  - Stage 1 on TensorE: e^T[a, row] accumulated over 18 K-chunks; tanh+bias
    fused into the PSUM eviction on ScalarE.
  - Scores via matmul with V as stationary; softmax on an [8, 50] tile.
  - alpha broadcast to 128 partitions via a K=1 matmul with a ones vector.
  - Stage 2 split between engines (VectorE reduce runs at 1 elem/cycle, so
    it alone would pace the kernel):
      * feature cols 0:512 on TensorE: alpha block-diagonal stationaries
        ([100, 2] per batch pair) against a natural-layout feature slice
        shipped from the host; accumulates straight into a PSUM bank in
        [b, f] layout.
      * feature cols 512:2048 on VectorE: context^T = reduce_r(X^T * alpha)
        over the same X^T tiles stage 1 uses.
  - context^T transposed back via PE at the end; outputs ctx f32 / alpha f32.
"""

import numpy as np
import ml_dtypes

import concourse.bass as bass
import concourse.tile as tile
import concourse.mybir as mybir
from concourse import bacc
from concourse.bass_utils import run_bass_kernel_spmd
from concourse.masks import make_identity

# Problem shapes (hardcoded per contract)
B, R, F, H, A = 512, 49, 2048, 256, 256
NCORES = 8
B_LOC = B // NCORES      # 64 batches per core
R_PAD = 50               # regions padded for alignment
ROWS = B_LOC * R_PAD     # 3200
K = F + H                # 2304 contraction (features + hidden fold-in)
KC = K // 128            # 18 K-chunks
FC = F // 128            # 16 feature chunks
FCP = 4                  # feature chunks whose stage-2 runs on the PE
FCD = FC - FCP           # remaining chunks on the DVE
FPE = FCP * 128          # 512 feature cols on the PE path
BLK_B = 8                # batches per pipeline block
BLK_ROWS = BLK_B * R_PAD # 400 = moving free dim
NBLK = B_LOC // BLK_B    # 8 blocks
NPAIR = BLK_B // 2       # batch pairs per block (K=100 rows each)

BF16 = mybir.dt.bfloat16
F32 = mybir.dt.float32

_CACHE = {}


def _build_module():
    nc = bacc.Bacc(
        "TRN2",
        target_bir_lowering=False,
        debug=False,
        num_devices=NCORES,
    )
    xt = nc.dram_tensor("xt", [K, ROWS], BF16, kind="ExternalInput").ap()
    xn = nc.dram_tensor("xn", [ROWS, FPE], BF16, kind="ExternalInput").ap()
    w = nc.dram_tensor("w", [K, A], BF16, kind="ExternalInput").ap()
    bias = nc.dram_tensor("bias", [128, 2], F32, kind="ExternalInput").ap()
    v = nc.dram_tensor("v", [128, 2], BF16, kind="ExternalInput").ap()
    ctx_out = nc.dram_tensor("ctx", [B_LOC, F], F32, kind="ExternalOutput").ap()
    alpha_out = nc.dram_tensor("alpha", [B_LOC, R], F32, kind="ExternalOutput").ap()

    with tile.TileContext(nc) as tc:
        _kernel_body(tc, xt, xn, w, bias, v, ctx_out, alpha_out)
    nc.compile()
    return nc


def _kernel_body(tc, xt, xn, w, bias, v, ctx_out, alpha_out):
    nc = tc.nc
    AX = mybir.AxisListType
    AF = mybir.ActivationFunctionType

    with (
        tc.tile_pool(name="const", bufs=1) as const_pool,
        tc.tile_pool(name="xtp", bufs=4) as xt_pool,
        tc.tile_pool(name="xnp", bufs=3) as xn_pool,
        tc.tile_pool(name="work", bufs=3) as work,
        tc.tile_pool(name="small", bufs=3) as small,
        tc.tile_pool(name="ps", bufs=2, space="PSUM") as psum,
    ):
        # ---- constants ----
        w_sb = const_pool.tile([128, KC, A], BF16)
        nc.sync.dma_start(w_sb[:], w.rearrange("(kc p) a -> p kc a", p=128))
        bias_sb = const_pool.tile([128, 2], F32)
        nc.sync.dma_start(bias_sb[:], bias)
        v_sb = const_pool.tile([128, 2], BF16)
        nc.sync.dma_start(v_sb[:], v)
        ones_sb = const_pool.tile([1, 128], BF16)
        nc.vector.memset(ones_sb[:], 1.0)
        ident = const_pool.tile([128, 128], BF16)
        make_identity(nc, ident[:])
        # context^T accumulator for the DVE path [f_in_chunk, (fcd, b)]
        ctxT = const_pool.tile([128, FCD, B_LOC], BF16)
        ctx_nat = const_pool.tile([B_LOC, FCD, 128], F32)
        # PE-path staging: pair p of block blk at partitions {32p,32p+1}, col blk
        cst = const_pool.tile([128, NBLK, FPE], F32)

        for blk in range(NBLK):
            xt_sb = xt_pool.tile([128, KC, BLK_ROWS], BF16)
            nc.sync.dma_start(
                xt_sb[:],
                xt.rearrange("(kc p) (nb n) -> p kc nb n", p=128, n=BLK_ROWS)[
                    :, :, blk, :
                ],
            )
            xn_sb = []
            for p in range(NPAIR):
                r0 = (blk * NPAIR + p) * 100
                xn_t = xn_pool.tile([100, FPE], BF16, tag=f"xn{p}")
                nc.sync.dma_start(xn_t[:], xn[r0:r0 + 100, :])
                xn_sb.append(xn_t)

            # ---- stage 1: e^T = tanh(W'^T X'^T + bias) ----
            e_sb = work.tile([128, 2, BLK_ROWS], BF16)
            for a0 in range(2):
                pe_ps = psum.tile([128, BLK_ROWS], F32, tag="pe", bufs=3)
                for kc in range(KC):
                    nc.tensor.matmul(
                        pe_ps[:],
                        lhsT=w_sb[:, kc, a0 * 128:(a0 + 1) * 128],
                        rhs=xt_sb[:, kc, :],
                        start=(kc == 0),
                        stop=(kc == KC - 1),
                    )
                nc.scalar.activation(
                    e_sb[:, a0, :], pe_ps[:], AF.Tanh,
                    bias=bias_sb[:, a0:a0 + 1], scale=1.0,
                )

            # ---- scores = V . e ----
            sc_ps = psum.tile([1, BLK_ROWS], F32, tag="sc", bufs=1)
            for a0 in range(2):
                nc.tensor.matmul(
                    sc_ps[:],
                    lhsT=v_sb[:, a0:a0 + 1],
                    rhs=e_sb[:, a0, :],
                    start=(a0 == 0),
                    stop=(a0 == 1),
                )
            sc_sb = small.tile([1, BLK_ROWS], F32)
            nc.scalar.copy(sc_sb[:], sc_ps[:])

            # ---- softmax over regions, on an [8, 50] layout ----
            sm = small.tile([BLK_B, R_PAD], F32)
            nc.sync.dma_start(sm[:], sc_sb[:])
            neg_mx = small.tile([BLK_B, 1], F32)
            nc.vector.tensor_reduce(
                neg_mx[:], sm[:, 0:R], axis=AX.X, op=mybir.AluOpType.max,
                negate=True,
            )
            ex = small.tile([BLK_B, R_PAD], F32)
            nc.scalar.activation(ex[:], sm[:], AF.Exp, bias=neg_mx[:], scale=1.0)
            den = small.tile([BLK_B, 1], F32)
            nc.vector.tensor_reduce(
                den[:], ex[:, 0:R], axis=AX.X, op=mybir.AluOpType.add
            )
            rec = small.tile([BLK_B, 1], F32)
            nc.vector.reciprocal(rec[:], den[:])
            al = small.tile([BLK_B, R], F32)
            nc.scalar.activation(al[:], ex[:, 0:R], AF.Copy, scale=rec[:])
            nc.sync.dma_start(alpha_out[blk * BLK_B:(blk + 1) * BLK_B, :], al[:])
            al_bf = small.tile([BLK_B, R_PAD], BF16)
            nc.vector.memset(al_bf[:, R:R_PAD], 0.0)
            nc.scalar.activation(al_bf[:, 0:R], ex[:, 0:R], AF.Copy, scale=rec[:])
            al_row = small.tile([1, BLK_ROWS], BF16)
            nc.sync.dma_start(al_row[:], al_bf[:])

            # ---- alpha^T [50, 8] and block-diag stationaries [100, 8] ----
            tp_al = psum.tile([R_PAD, BLK_B], BF16, tag="tpal", bufs=1)
            nc.tensor.transpose(tp_al[:], al_bf[:], ident[0:BLK_B, 0:BLK_B])
            alT = small.tile([R_PAD, BLK_B], BF16)
            nc.scalar.copy(alT[:], tp_al[:])
            s_bd = small.tile([2 * R_PAD, BLK_B], BF16)
            nc.vector.memset(s_bd[:], 0.0)
            nc.sync.dma_start(s_bd[0:R_PAD, 0:BLK_B:2], alT[:, 0:BLK_B:2])
            nc.sync.dma_start(s_bd[R_PAD:2 * R_PAD, 1:BLK_B:2], alT[:, 1:BLK_B:2])

            # ---- stage 2a (PE): ctx[b, 0:512] via block-diag matmuls ----
            # pair p lands at PSUM partitions {32p, 32p+1} (col tile_position)
            ctx_pb = psum.tile([128, FPE], F32, tag="ctxp", bufs=1)
            for p in range(NPAIR):
                nc.tensor.matmul(
                    ctx_pb[32 * p:32 * p + 2, :],
                    lhsT=s_bd[:, 2 * p:2 * p + 2],
                    rhs=xn_sb[p][:, :],
                    start=True,
                    stop=True,
                    tile_position=(0, 32 * p),
                )
            for p in range(NPAIR):
                nc.scalar.copy(
                    cst[32 * p:32 * p + 2, blk, :],
                    ctx_pb[32 * p:32 * p + 2, :],
                )

            # ---- broadcast alpha across partitions via K=1 matmul ----
            ab_ps = psum.tile([128, BLK_ROWS], F32, tag="ab", bufs=1)
            nc.tensor.matmul(
                ab_ps[:], lhsT=ones_sb[:], rhs=al_row[:], start=True, stop=True
            )
            ab_sb = work.tile([128, BLK_ROWS], BF16)
            nc.scalar.copy(ab_sb[:], ab_ps[:])

            # ---- stage 2b (DVE): context^T for f 512:2048 ----
            tmp = work.tile([128, FCD, BLK_ROWS], BF16)
            nc.vector.tensor_mul(
                tmp[:],
                xt_sb[:, FCP:FC, :],
                ab_sb.rearrange("p (x n) -> p x n", x=1).broadcast_to(
                    [128, FCD, BLK_ROWS]
                ),
            )
            with nc.allow_low_precision("context emitted once in bf16"):
                nc.vector.tensor_reduce(
                    ctxT[:, :, blk * BLK_B:(blk + 1) * BLK_B],
                    tmp.rearrange("p fc (b r) -> p fc b r", b=BLK_B),
                    axis=AX.X,
                    op=mybir.AluOpType.add,
                )

        # ---- PE-path context out: partition-scatter DMA per pair slot ----
        ctx_bb = ctx_out.rearrange("(nb bb) f -> bb nb f", bb=BLK_B)
        for p in range(NPAIR):
            nc.sync.dma_start(
                ctx_bb[2 * p:2 * p + 2, :, 0:FPE], cst[32 * p:32 * p + 2, :, :]
            )
        # ---- DVE-path context: transpose back per chunk ----
        for kc in range(FCD):
            tp_ps = psum.tile([64, 128], BF16, tag="tp", bufs=1)
            nc.tensor.transpose(tp_ps[:], ctxT[:, kc, :], ident[:])
            nc.scalar.copy(ctx_nat[:, kc, :], tp_ps[:])
        nc.sync.dma_start(
            ctx_out[:, FPE:F], ctx_nat.rearrange("b fc f -> b (fc f)")
        )


def _prep_inputs(features, hidden, W_feat, b_feat, W_hidden, b_hidden, V_w, V_b):
    """Host-side shard + pack. Returns in_maps for the 8 cores."""
    bf = ml_dtypes.bfloat16
    w_all = np.concatenate([W_feat, W_hidden], axis=0).astype(bf)       # [K, A]
    bias_all = (b_feat + b_hidden).astype(np.float32)                   # [A]
    bias_pk = np.ascontiguousarray(bias_all.reshape(2, 128).T)          # [128, 2]
    v_pk = np.ascontiguousarray(V_w.astype(bf).reshape(2, 128).T)       # [128, 2]

    in_maps = []
    for c in range(NCORES):
        fs = features[c * B_LOC:(c + 1) * B_LOC]    # [64, 49, 2048] f32
        hs = hidden[c * B_LOC:(c + 1) * B_LOC]      # [64, 256] f32
        fs_bf = fs.astype(bf)
        xt = np.zeros((K, B_LOC, R_PAD), dtype=bf)
        xt[:F, :, :R] = fs_bf.transpose(2, 0, 1)
        xt[F:, :, :] = hs.T.astype(bf)[:, :, None]
        xn = np.zeros((B_LOC, R_PAD, FPE), dtype=bf)
        xn[:, :R, :] = fs_bf[:, :, :FPE]
        in_maps.append({
            "xt": np.ascontiguousarray(xt.reshape(K, ROWS)),
            "xn": np.ascontiguousarray(xn.reshape(ROWS, FPE)),
            "w": w_all,
            "bias": bias_pk,
            "v": v_pk,
        })
    return in_maps


def run(inputs, trace=False):
    """Compile (cached), execute on 8 cores, gather. Returns dict with
    full-shape outputs and profiling info."""
    if "nc" not in _CACHE:
        _CACHE["nc"] = _build_module()
    nc = _CACHE["nc"]
    in_maps = _prep_inputs(**inputs)
    res = run_bass_kernel_spmd(
        nc, in_maps, core_ids=list(range(NCORES)), trace=trace
    )
    ctx = np.concatenate([r["ctx"] for r in res.results], axis=0)
    alpha = np.concatenate([r["alpha"] for r in res.results], axis=0)
    return {
        "context": ctx.astype(np.float32),
        "alpha": alpha.astype(np.float32),
        "exec_time_ns": res.exec_time_ns,
        "profile_json": res.profile_json,
    }


def kernel(**inputs):
    out = run(inputs, trace=False)
    return out["context"], out["alpha"]


# revision 28
# speedup vs baseline: 1.2689x; 1.0741x over previous
"""Additive-attention kernel for one TRN2 chip (8 NeuronCores, data parallel).

Computes, per batch element b:
    e      = tanh(features @ W_feat + hidden @ W_hidden + b_feat + b_hidden)
    scores = e @ V_w                (V_b dropped: softmax is shift-invariant)
    alpha  = softmax(scores, axis=regions)
    context= sum_r alpha[b,r] * features[b,r,:]

Device strategy (per core, 64 batches):
  - Host pre-packs X'^T = [features; hidden]^T in bf16 with regions padded
    49->50, so the contraction dim (2304 = 2048+256) sits on partitions.
  - Stage 1 on TensorE: e^T[a, row] accumulated over 18 K-chunks; tanh+bias
    fused into the PSUM eviction on ScalarE.
  - Scores via matmul with V as stationary; softmax on a [batch, 50] tile.
  - alpha broadcast to 128 partitions via a K=1 matmul with a ones vector.
  - Stage 2 split between engines (VectorE reduce runs at 1 elem/cycle, so
    it alone would pace the kernel):
      * feature cols 0:768 on TensorE: alpha block-diagonal stationaries
        ([100, 2] per batch pair) against a natural-layout feature slice
        shipped from the host; pairs land at 32-aligned PSUM partitions
        via tile_position.
      * feature cols 768:2048 on VectorE: context^T = reduce_r(X^T * alpha)
        over the same X^T tiles stage 1 uses.
  - Batches processed in pipeline blocks of [4,4,8,8,8,8,8,8,4,4] to keep
    the pipeline fill/drain short; context^T transposed back via PE.
"""

import numpy as np
import ml_dtypes

import concourse.bass as bass
import concourse.tile as tile
import concourse.mybir as mybir
from concourse import bacc
from concourse.bass_utils import run_bass_kernel_spmd
from concourse.masks import make_identity

# Problem shapes (hardcoded per contract)
B, R, F, H, A = 512, 49, 2048, 256, 256
NCORES = 8
B_LOC = B // NCORES      # 64 batches per core
R_PAD = 50               # regions padded for alignment
ROWS = B_LOC * R_PAD     # 3200
K = F + H                # 2304 contraction (features + hidden fold-in)
KC = K // 128            # 18 K-chunks
FC = F // 128            # 16 feature chunks
FCP = 4                  # feature chunks whose stage-2 runs on the PE
FCD = FC - FCP           # remaining chunks on the DVE
FPE = FCP * 128          # feature cols on the PE path
BLOCK_SIZES = [8] * 8   # batches per pipeline block

BF16 = mybir.dt.bfloat16
F32 = mybir.dt.float32

_CACHE = {}


def _build_module():
    nc = bacc.Bacc(
        "TRN2",
        target_bir_lowering=False,
        debug=False,
        num_devices=NCORES,
    )
    xt = nc.dram_tensor("xt", [K, ROWS], BF16, kind="ExternalInput").ap()
    xn = nc.dram_tensor("xn", [ROWS, FPE], BF16, kind="ExternalInput").ap()
    w = nc.dram_tensor("w", [K, A], BF16, kind="ExternalInput").ap()
    bias = nc.dram_tensor("bias", [128, 2], F32, kind="ExternalInput").ap()
    v = nc.dram_tensor("v", [128, 2], BF16, kind="ExternalInput").ap()
    ctx_out = nc.dram_tensor("ctx", [B_LOC, F], F32, kind="ExternalOutput").ap()
    alpha_out = nc.dram_tensor("alpha", [B_LOC, R], F32, kind="ExternalOutput").ap()

    with tile.TileContext(nc) as tc:
        _kernel_body(tc, xt, xn, w, bias, v, ctx_out, alpha_out)
    nc.compile()
    return nc


def _kernel_body(tc, xt, xn, w, bias, v, ctx_out, alpha_out):
    nc = tc.nc
    AX = mybir.AxisListType
    AF = mybir.ActivationFunctionType

    with (
        tc.tile_pool(name="const", bufs=1) as const_pool,
        tc.tile_pool(name="xtp", bufs=5) as xt_pool,
        tc.tile_pool(name="xnp", bufs=4) as xn_pool,
        tc.tile_pool(name="work", bufs=4) as work,
        tc.tile_pool(name="small", bufs=6) as small,
        tc.tile_pool(name="ps", bufs=2, space="PSUM") as psum,
        tc.tile_pool(name="dr", bufs=3, space="DRAM") as dram_pool,
    ):
        # ---- constants ----
        w_sb = const_pool.tile([128, KC, A], BF16)
        w_v = w.rearrange("(kc p) a -> p kc a", p=128)
        nc.sync.dma_start(w_sb[:, 0:3, :], w_v[:, 0:3, :])
        nc.sync.dma_start(w_sb[:, 3:KC, :], w_v[:, 3:KC, :])
        bias_sb = const_pool.tile([128, 2], F32)
        nc.sync.dma_start(bias_sb[:], bias)
        v_sb = const_pool.tile([128, 2], BF16)
        nc.sync.dma_start(v_sb[:], v)
        ones_sb = const_pool.tile([1, 128], BF16)
        nc.vector.memset(ones_sb[:], 1.0)
        ident = const_pool.tile([128, 128], BF16)
        make_identity(nc, ident[:])
        # context^T accumulator for the DVE path [f_in_chunk, (fcd, b)]
        ctxT = const_pool.tile([128, FCD, B_LOC], BF16)
        ctx_nat = const_pool.tile([B_LOC, FCD, 128], F32)
        # PE-path staging: global pair gp -> partitions {32*(gp%4)..+1}, col gp//4
        cst = const_pool.tile([128, B_LOC // 8, FPE], F32)
        xt_v = xt.rearrange("(kc p) r -> p kc r", p=128)
        # persistent pair-MM PSUM bank; lanes {32p..32p+1} hold live pairs
        ctx_pb = psum.tile([128, 512], F32, tag="ctxp", bufs=1)
        nc.vector.memset(ctx_pb[:], 0.0)

        bo = 0
        for nb in BLOCK_SIZES:
            r0, rows_b = bo * R_PAD, nb * R_PAD
            npair = nb // 2
            xt_sb = xt_pool.tile([128, KC, 8 * R_PAD], BF16)
            if bo == 0:
                nc.sync.dma_start(
                    xt_sb[:, 0:3, 0:rows_b], xt_v[:, 0:3, r0:r0 + rows_b]
                )
                nc.sync.dma_start(
                    xt_sb[:, 3:9, 0:rows_b], xt_v[:, 3:9, r0:r0 + rows_b]
                )
            else:
                nc.sync.dma_start(
                    xt_sb[:, 0:9, 0:rows_b], xt_v[:, 0:9, r0:r0 + rows_b]
                )
            nc.scalar.dma_start(
                xt_sb[:, 9:KC, 0:rows_b], xt_v[:, 9:KC, r0:r0 + rows_b]
            )
            gp0 = bo // 2
            xn_t = xn_pool.tile([100, 4, FPE], BF16)
            nc.sync.dma_start(
                xn_t[:, 0:npair, :],
                xn.rearrange("(gp pr) f -> pr gp f", pr=100)[:, gp0:gp0 + npair, :],
            )

            # ---- stage 1: e^T = tanh(W'^T X'^T + bias) ----
            e_sb = work.tile([128, 2, 8 * R_PAD], BF16)
            for a0 in range(2):
                pe_ps = psum.tile([128, 8 * R_PAD], F32, tag="pe", bufs=4)
                for kc in range(KC):
                    nc.tensor.matmul(
                        pe_ps[:, 0:rows_b],
                        lhsT=w_sb[:, kc, a0 * 128:(a0 + 1) * 128],
                        rhs=xt_sb[:, kc, 0:rows_b],
                        start=(kc == 0),
                        stop=(kc == KC - 1),
                    )
                nc.scalar.activation(
                    e_sb[:, a0, 0:rows_b], pe_ps[:, 0:rows_b], AF.Tanh,
                    bias=bias_sb[:, a0:a0 + 1], scale=1.0,
                )

            # ---- scores = V . e ----
            sc_ps = psum.tile([1, 8 * R_PAD], F32, tag="sc", bufs=1)
            for a0 in range(2):
                nc.tensor.matmul(
                    sc_ps[:, 0:rows_b],
                    lhsT=v_sb[:, a0:a0 + 1],
                    rhs=e_sb[:, a0, 0:rows_b],
                    start=(a0 == 0),
                    stop=(a0 == 1),
                )
            sc_sb = small.tile([1, 8 * R_PAD], F32)
            nc.scalar.copy(sc_sb[:, 0:rows_b], sc_ps[:, 0:rows_b])

            # ---- softmax over regions, on an [nb, 50] layout ----
            sm = small.tile([8, R_PAD], F32)
            nc.gpsimd.dma_start(sm[0:nb, :], sc_sb[:, 0:rows_b])
            neg_mx = small.tile([8, 1], F32)
            nc.vector.tensor_reduce(
                neg_mx[0:nb, :], sm[0:nb, 0:R], axis=AX.X,
                op=mybir.AluOpType.max, negate=True,
            )
            ex = small.tile([8, R_PAD], F32)
            nc.scalar.activation(
                ex[0:nb, :], sm[0:nb, :], AF.Exp, bias=neg_mx[0:nb, :], scale=1.0
            )
            den = small.tile([8, 1], F32)
            nc.vector.tensor_reduce(
                den[0:nb, :], ex[0:nb, 0:R], axis=AX.X, op=mybir.AluOpType.add
            )
            rec = small.tile([8, 1], F32)
            nc.vector.reciprocal(rec[0:nb, :], den[0:nb, :])
            al = small.tile([8, R], F32)
            nc.scalar.activation(
                al[0:nb, :], ex[0:nb, 0:R], AF.Copy, scale=rec[0:nb, :]
            )
            nc.gpsimd.dma_start(alpha_out[bo:bo + nb, :], al[0:nb, :])
            al_bf = small.tile([8, R_PAD], BF16)
            nc.vector.memset(al_bf[0:nb, R:R_PAD], 0.0)
            nc.scalar.activation(
                al_bf[0:nb, 0:R], ex[0:nb, 0:R], AF.Copy, scale=rec[0:nb, :]
            )
            al_row = small.tile([1, 8 * R_PAD], BF16)
            nc.gpsimd.dma_start(al_row[:, 0:rows_b], al_bf[0:nb, :])

            # ---- alpha^T [50, nb] and block-diag stationaries [100, nb] ----
            tp_al = psum.tile([R_PAD, 8], BF16, tag="tp", bufs=1)
            nc.tensor.transpose(
                tp_al[:, 0:nb], al_bf[0:nb, :], ident[0:nb, 0:nb]
            )
            alT = small.tile([R_PAD, 8], BF16)
            nc.scalar.copy(alT[:, 0:nb], tp_al[:, 0:nb])
            s_bd = small.tile([2 * R_PAD, 8], BF16)
            nc.vector.memset(s_bd[:, 0:nb], 0.0)
            nc.gpsimd.dma_start(s_bd[0:R_PAD, 0:nb:2], alT[:, 0:nb:2])
            nc.gpsimd.dma_start(s_bd[R_PAD:2 * R_PAD, 1:nb:2], alT[:, 1:nb:2])

            # ---- stage 2a (PE): ctx[b, 0:FPE] via block-diag matmuls ----
            for p in range(npair):
                gp = bo // 2 + p
                lane = gp % 4
                for fg in range((FPE + 511) // 512):
                    f0 = fg * 512
                    f1 = min(FPE, f0 + 512)
                    nc.tensor.matmul(
                        ctx_pb[32 * lane:32 * lane + 2, f0:f1],
                        lhsT=s_bd[:, 2 * p:2 * p + 2],
                        rhs=xn_t[:, p, f0:f1],
                        start=True,
                        stop=True,
                        tile_position=(0, 32 * lane),
                    )

            nc.scalar.copy(cst[:, (bo // 2) // 4, :], ctx_pb[:])

            # ---- broadcast alpha across partitions via K=1 matmul ----
            ab_ps = psum.tile([128, 8 * R_PAD], F32, tag="ab", bufs=1)
            nc.tensor.matmul(
                ab_ps[:, 0:rows_b], lhsT=ones_sb[:], rhs=al_row[:, 0:rows_b],
                start=True, stop=True,
            )
            ab_sb = work.tile([128, 8 * R_PAD], BF16)
            nc.scalar.copy(ab_sb[:, 0:rows_b], ab_ps[:, 0:rows_b])

            # ---- stage 2b (DVE): context^T for f FPE:2048 ----
            tmp = work.tile([128, FCD, 8 * R_PAD], BF16)
            nc.vector.tensor_mul(
                tmp[:, :, 0:rows_b],
                xt_sb[:, FCP:FC, 0:rows_b],
                ab_sb[:, 0:rows_b].rearrange("p (x n) -> p x n", x=1).broadcast_to(
                    [128, FCD, rows_b]
                ),
            )
            with nc.allow_low_precision("context emitted once in bf16"):
                nc.vector.tensor_reduce(
                    ctxT[:, :, bo:bo + nb],
                    tmp[:, :, 0:rows_b].rearrange("p fc (b r) -> p fc b r", b=nb),
                    axis=AX.X,
                    op=mybir.AluOpType.add,
                )
            bo += nb

        # ---- PE-path context out: partition-scatter DMA per pair lane ----
        ctx_bb = ctx_out.rearrange("(nb bb) f -> bb nb f", bb=8)
        for lane in range(4):
            nc.sync.dma_start(
                ctx_bb[2 * lane:2 * lane + 2, :, 0:FPE],
                cst[32 * lane:32 * lane + 2, :, :],
            )
        # ---- DVE-path context: transpose back per chunk ----
        tp_all = psum.tile([128, (FCD + 1) // 2, 128], BF16, tag="ctxp", bufs=1)
        for kc in range(FCD):
            half, col = kc % 2, kc // 2
            nc.tensor.transpose(
                tp_all[64 * half:64 * half + 64, col, :],
                ctxT[:, kc, :], ident[:],
                tile_position=(0, 64 * half),
            )
            nc.scalar.copy(ctx_nat[:, kc, :], tp_all[64 * half:64 * half + 64, col, :])
        nc.sync.dma_start(
            ctx_out[:, FPE:F], ctx_nat.rearrange("b fc f -> b (fc f)")
        )


def _prep_inputs(features, hidden, W_feat, b_feat, W_hidden, b_hidden, V_w, V_b):
    """Host-side shard + pack. Returns in_maps for the 8 cores."""
    bf = ml_dtypes.bfloat16
    w_all = np.concatenate([W_feat, W_hidden], axis=0).astype(bf)       # [K, A]
    bias_all = (b_feat + b_hidden).astype(np.float32)                   # [A]
    bias_pk = np.ascontiguousarray(bias_all.reshape(2, 128).T)          # [128, 2]
    v_pk = np.ascontiguousarray(V_w.astype(bf).reshape(2, 128).T)       # [128, 2]

    in_maps = []
    for c in range(NCORES):
        fs = features[c * B_LOC:(c + 1) * B_LOC]    # [64, 49, 2048] f32
        hs = hidden[c * B_LOC:(c + 1) * B_LOC]      # [64, 256] f32
        fs_bf = fs.astype(bf)
        xt = np.zeros((K, B_LOC, R_PAD), dtype=bf)
        xt[:F, :, :R] = fs_bf.transpose(2, 0, 1)
        xt[F:, :, :] = hs.T.astype(bf)[:, :, None]
        xn = np.zeros((B_LOC, R_PAD, FPE), dtype=bf)
        xn[:, :R, :] = fs_bf[:, :, :FPE]
        in_maps.append({
            "xt": np.ascontiguousarray(xt.reshape(K, ROWS)),
            "xn": np.ascontiguousarray(xn.reshape(ROWS, FPE)),
            "w": w_all,
            "bias": bias_pk,
            "v": v_pk,
        })
    return in_maps


def run(inputs, trace=False):
    """Compile (cached), execute on 8 cores, gather. Returns dict with
    full-shape outputs and profiling info."""
    if "nc" not in _CACHE:
        _CACHE["nc"] = _build_module()
    nc = _CACHE["nc"]
    in_maps = _prep_inputs(**inputs)
    res = run_bass_kernel_spmd(
        nc, in_maps, core_ids=list(range(NCORES)), trace=trace
    )
    ctx = np.concatenate([r["ctx"] for r in res.results], axis=0)
    alpha = np.concatenate([r["alpha"] for r in res.results], axis=0)
    return {
        "context": ctx.astype(np.float32),
        "alpha": alpha.astype(np.float32),
        "exec_time_ns": res.exec_time_ns,
        "profile_json": res.profile_json,
    }


def kernel(**inputs):
    out = run(inputs, trace=False)
    return out["context"], out["alpha"]
